# revision 16
# baseline (speedup 1.0000x reference)
"""Trainium2 Bass kernel for nn_Attention_35021163332119.

Full multi-head attention: qkv = x @ w_qkv; RoPE(q, k); softmax(q k^T / sqrt(dh)) v;
out = heads @ w_out + b_out.  B=2, N=2048, DIM=1024, H=16, DH=64.

Sharding: 8 cores = (batch b in {0,1}) x (head-group g in {0..3} of 4 heads).
Each core computes its 4 heads end-to-end plus the partial output projection
for its head-group's rows of w_out; the host sums the 4 partials per batch
and adds b_out.

On-core layout: x is host-transposed to xT [DIM, N] so the contraction dim
sits on SBUF partitions.  q,k are produced transposed ([dh, n], head pairs
stacked on 128 partitions) straight out of the QKV matmul; v is produced in
natural [n, dh] layout with an extra ones column, so the PV matmul (M=65)
also accumulates the softmax denominator in row 64.  RoPE's interleaved
pair-rotation is a 128x128 +/-1 permutation matmul on the PE plus two DVE
multiplies against cos/sin tables.

All matmuls run in bf16 (full PE stream rate; fp32r streams at half rate on
HW).  Scores accumulate fp32 in PSUM; exp runs on the Scalar engine reading
PSUM directly.  Per j-tile a single PSUM tile holds BOTH heads' scores side
by side so one ACTIVATE covers both heads and only one ps buffer is
consumed per j-step.

Schedule: the attention phase is Scalar(exp)-bound (~1.16us/j-step vs
~0.65us of PE work), so only K (both pairs), V tiles 0-7 and the first Q
chunk are computed up front; the remaining Q chunks, V tiles 8-15 and the
output projection are emitted as small "filler" pieces inside the attention
blocks' j loops where the PE has slack.  Inputs are loaded with a handful
of batched DMAs (multi-tile interleaved layout) because each DMA dispatch
costs ~600ns on the sync engine.
"""

import numpy as np

B, N, DIM, H, DH = 2, 2048, 1024, 16, 64
ROPE_BASE = 10000.0
SCALE = DH ** -0.5
N_CORES = 8
G = 4                 # heads per core
KT = DIM // 128       # contraction tiles
NT = N // 128         # sequence tiles

_cache = {}


def _rope_tables():
    inv_freq = (1.0 / (ROPE_BASE ** (np.arange(0, DH, 2, dtype=np.float32) / DH)))
    t = np.arange(N, dtype=np.float32)
    freqs = t[:, None] * inv_freq[None, :]          # [N, DH/2]
    freqs = np.repeat(freqs, 2, axis=-1)            # [N, DH] interleaved
    cosT = np.cos(freqs).T.astype(np.float32)       # [DH, N]
    sinT = np.sin(freqs).T.astype(np.float32)
    cos2 = np.concatenate([cosT, cosT], axis=0)     # [128, N] two heads stacked
    sin2 = np.concatenate([sinT, sinT], axis=0)
    return np.ascontiguousarray(cos2), np.ascontiguousarray(sin2)


def _p2t():
    # rot = P2 @ qT with P2 = blockdiag(P, P), P[2t, 2t+1] = -1, P[2t+1, 2t] = 1
    # matmul computes lhsT.T @ rhs, so pass P2.T
    p = np.zeros((DH, DH), dtype=np.float32)
    for t in range(DH // 2):
        p[2 * t, 2 * t + 1] = -1.0
        p[2 * t + 1, 2 * t] = 1.0
    p2 = np.zeros((128, 128), dtype=np.float32)
    p2[:DH, :DH] = p
    p2[DH:, DH:] = p
    return np.ascontiguousarray(p2.T)


def _build():
    if "nc" in _cache:
        return _cache["nc"]

    import concourse.mybir as mybir
    import concourse.tile as tile
    from concourse import bacc

    F32 = mybir.dt.float32
    F32R = mybir.dt.float32r
    BF16 = mybir.dt.bfloat16
    EXP = mybir.ActivationFunctionType.Exp

    nc = bacc.Bacc("TRN2", target_bir_lowering=False, debug=False)
    xT_d = nc.dram_tensor("xT", [DIM, N], BF16, kind="ExternalInput")
    wqk_d = nc.dram_tensor("wqk", [DIM, 4 * 128], BF16, kind="ExternalInput")
    wv_d = nc.dram_tensor("wv", [DIM, G * DH], BF16, kind="ExternalInput")
    wout_d = nc.dram_tensor("wout", [G * DH, DIM], BF16, kind="ExternalInput")
    cos_d = nc.dram_tensor("cos2", [128, N], BF16, kind="ExternalInput")
    sin_d = nc.dram_tensor("sin2", [128, N], BF16, kind="ExternalInput")
    p2t_d = nc.dram_tensor("p2t", [128, 128], BF16, kind="ExternalInput")
    ones_d = nc.dram_tensor("onesr", [DH + 1, DH], mybir.dt.float32,
                            kind="ExternalInput")
    part_d = nc.dram_tensor("part", [N, DIM], BF16, kind="ExternalOutput")

    with tile.TileContext(nc) as tc:
        with tc.tile_pool(name="persist", bufs=1) as persist, \
             tc.tile_pool(name="att", bufs=5) as att, \
             tc.tile_pool(name="norm_w", bufs=2) as norm_w, \
             tc.tile_pool(name="outp", bufs=3) as outp, \
             tc.tile_pool(name="xph", bufs=1) as xph, \
             tc.tile_pool(name="rope_w", bufs=2) as rope_w, \
             tc.tile_pool(name="ps", bufs=3, space="PSUM") as ps, \
             tc.tile_pool(name="pso", bufs=2, space="PSUM") as pso:

            # ---- persistent tiles ----
            qk_sb = [persist.tile([128, N], BF16, tag=f"qk{m}", name=f"qk{m}")
                     for m in range(4)]          # q01T, q23T, k01T, k23T
            v_aug = [persist.tile([128, G, DH + 1], BF16, tag=f"vaug{tn}",
                                  name=f"vaug{tn}")
                     for tn in range(NT)]        # per-j-tile for precise deps
            wout_sb = persist.tile([128, 2, DIM], BF16, tag="wo", name="wo")
            wout_hi2 = persist.tile([DH, DIM], BF16, tag="wohi", name="wohi")
            tmpb33 = persist.tile([DH, 512], BF16, tag="tmpb33", name="tmpb33")
            outT = [[persist.tile([128, 512], BF16, tag=f"outT{p}_{iq}",
                                  name=f"outT{p}_{iq}")
                     for iq in range(4)] for p in range(2)]

            # ---- phase-1 tiles (multi-tile interleaved so one DMA loads all
            # k-tiles of a column chunk) ----
            xT = xph.tile([128, KT, N], BF16, tag="xT", name="xT")
            wqk = xph.tile([128, KT, 4 * 128], BF16, tag="wqk", name="wqk")
            wv = xph.tile([128, KT, G * DH], BF16, tag="wv", name="wv")
            cos2 = xph.tile([128, N], BF16, tag="cos2")
            sin2 = xph.tile([128, N], BF16, tag="sin2")
            p2t = xph.tile([128, 128], BF16, tag="p2t")
            ones_r = xph.tile([DH + 1, DH], F32R, tag="ones_r")

            # ---- input DMA, priority order, batched ----
            xT_r = xT_d.ap().rearrange("(t p) n -> p t n", p=128)
            wqk_r = wqk_d.ap().rearrange("(t p) m -> p t m", p=128)
            nc.sync.dma_start(out=wqk[:, :, 256:512], in_=wqk_r[:, :, 256:512])
            nc.sync.dma_start(out=xT[:, 0:4, 0:512], in_=xT_r[:, 0:4, 0:512])
            nc.sync.dma_start(out=xT[:, 4:8, 0:512], in_=xT_r[:, 4:8, 0:512])
            nc.sync.dma_start(out=xT[:, :, 512:1024], in_=xT_r[:, :, 512:1024])
            nc.sync.dma_start(
                out=wv, in_=wv_d.ap().rearrange("(t p) m -> p t m", p=128))
            nc.sync.dma_start(out=cos2, in_=cos_d.ap())
            nc.sync.dma_start(out=sin2, in_=sin_d.ap())
            nc.sync.dma_start(out=p2t, in_=p2t_d.ap())
            nc.sync.dma_start(out=ones_r, in_=ones_d.ap().bitcast(F32R))
            nc.sync.dma_start(out=wqk[:, :, 0:256], in_=wqk_r[:, :, 0:256])
            nc.sync.dma_start(out=xT[:, :, 1024:1536], in_=xT_r[:, :, 1024:1536])
            nc.sync.dma_start(out=xT[:, :, 1536:2048], in_=xT_r[:, :, 1536:2048])
            nc.sync.dma_start(
                out=wout_sb, in_=wout_d.ap().rearrange("(t p) m -> p t m", p=128))
            nc.sync.dma_start(
                out=wout_hi2,
                in_=wout_d.ap().rearrange("(t p) m -> p t m", p=64)[:, 3, :])
            for tn in range(NT):
                nc.vector.memset(v_aug[tn][:, :, DH:DH + 1], 1.0)

            # ---- emitters ----
            def qk_chunk_mm(m, c2, half, klo, khi, holder):
                """Piece of the [128,1024] QKV chain for tile m, chunk c2:
                k-range [klo,khi) of the `half` 512-accumulation."""
                if holder.get("t") is None:
                    holder["t"] = ps.tile([128, 1024], F32, tag="s",
                                          name=f"mm_qk{m}_{c2}")
                mm_ps = holder["t"]
                hsl = slice(half * 512, (half + 1) * 512)
                csl = slice(c2 * 1024 + half * 512, c2 * 1024 + (half + 1) * 512)
                for k in range(klo, khi):
                    nc.tensor.matmul(
                        mm_ps[:, hsl],
                        wqk[:, k, m * 128:(m + 1) * 128],
                        xT[:, k, csl],
                        start=(k == 0), stop=(k == KT - 1))

            def qk_chunk_finish(m, c2, holder, use_vector):
                csl = slice(c2 * 1024, (c2 + 1) * 1024)
                if use_vector:
                    nc.vector.tensor_copy(qk_sb[m][:, csl], holder["t"])
                else:
                    nc.scalar.copy(qk_sb[m][:, csl], holder["t"])
                holder["t"] = None

            def rope_rot(m, c2, holder):
                """rot = P2 @ qk chunk -> PSUM."""
                holder["t"] = ps.tile([128, 1024], F32, tag="s",
                                      name=f"mm_rot{m}_{c2}")
                for half in range(2):
                    csl = slice(c2 * 1024 + half * 512,
                                c2 * 1024 + (half + 1) * 512)
                    nc.tensor.matmul(
                        holder["t"][:, half * 512:(half + 1) * 512],
                        p2t, qk_sb[m][:, csl],
                        start=True, stop=True)

            def rope_finish(m, c2, holder):
                csl = slice(c2 * 1024, (c2 + 1) * 1024)
                tmp = rope_w.tile([128, 1024], BF16, tag="ropetmp")
                nc.vector.tensor_mul(tmp, holder["t"], sin2[:, csl])
                nc.vector.tensor_mul(qk_sb[m][:, csl], qk_sb[m][:, csl],
                                     cos2[:, csl])
                nc.vector.tensor_add(qk_sb[m][:, csl], qk_sb[m][:, csl], tmp)
                holder["t"] = None

            def v_tile(tn):
                mm_ps = ps.tile([128, 1024], F32, tag="s", name=f"mm_v{tn}")
                for k in range(KT):
                    nc.tensor.matmul(
                        mm_ps[:, 0:G * DH],
                        xT[:, k, tn * 128:(tn + 1) * 128],
                        wv[:, k, :],
                        start=(k == 0), stop=(k == KT - 1))
                nc.vector.tensor_copy(
                    v_aug[tn][:, :, 0:DH],
                    mm_ps[:, 0:G * DH].rearrange("p (h d) -> p h d", h=G))

            def proj_tile(tn, copy_eng):
                nsl = slice((tn % 4) * 128, (tn % 4) * 128 + 128)
                iq = tn // 4
                f_ps = ps.tile([128, 1024], F32, tag="s", name=f"f_ps{tn}")
                for c2 in range(2):
                    c2sl = slice(c2 * 512, (c2 + 1) * 512)
                    for kk in range(2):
                        nc.tensor.matmul(
                            f_ps[:, c2sl],
                            outT[kk][iq][:, nsl], wout_sb[:, kk, c2sl],
                            start=(kk == 0), stop=(kk == 1))
                out_sb = outp.tile([128, DIM], BF16, tag="osb")
                copy_eng(out_sb, f_ps)
                nc.sync.dma_start(
                    out=part_d.ap().rearrange("(t p) m -> t p m", p=128)[tn],
                    in_=out_sb)

            def proj_last(tn, copy_eng):
                nsl = slice((tn % 4) * 128, (tn % 4) * 128 + 128)
                f_ps = ps.tile([128, 1024], F32, tag="s", name=f"f_ps{tn}")
                for c2 in range(2):
                    c2sl = slice(c2 * 512, (c2 + 1) * 512)
                    nc.tensor.matmul(
                        f_ps[:, c2sl], outT[0][3][:, nsl],
                        wout_sb[:, 0, c2sl], start=True, stop=False)
                    nc.tensor.matmul(
                        f_ps[:, c2sl], outT[1][3][0:DH, nsl],
                        wout_sb[0:DH, 1, c2sl], start=False, stop=False)
                    nc.tensor.matmul(
                        f_ps[:, c2sl], tmpb33[:, nsl],
                        wout_hi2[:, c2sl], start=False, stop=True)
                out_sb = outp.tile([128, DIM], BF16, tag="osb")
                copy_eng(out_sb, f_ps)
                nc.sync.dma_start(
                    out=part_d.ap().rearrange("(t p) m -> t p m", p=128)[tn],
                    in_=out_sb)

            def attention(p, iq, fillers=None, start_j=1, pre=None,
                          last=False):
                """One (head-pair, i-quarter of 512) block.  Per j-tile a
                single PSUM tile holds BOTH heads' scores side by side
                ([j=128, head0 i | head1 i]) so one ACTIVATE covers both
                heads and only one ps buffer is consumed per j-step - the
                scores pipeline keeps a 2-step cushion even when a filler
                chain occupies a third buffer.  `fillers` is a list of
                zero-arg closures emitting small PE pieces into the PE
                slack (the block is Scalar-bound); one is drained every
                second j-step starting at `start_j` (delay it if the filler
                depends on the previous block's normalization).  `pre` is
                the previous block's norm tail, emitted at j==1 so the PE
                does not stall on it at the block boundary.  Returns this
                block's norm tail closure."""
                fillers = list(fillers or [])
                qT = qk_sb[p]
                kTt = qk_sb[2 + p]
                i0 = iq * 512
                isl = slice(i0, i0 + 512)
                o_ps = [pso.tile([DH + 1, 512], F32, tag="o", name=f"o{hh}")
                        for hh in range(2)]

                def emit_pv(j, expT):
                    for hh in range(2):
                        nc.tensor.matmul(
                            o_ps[hh],
                            v_aug[j][:, 2 * p + hh, :],
                            expT[:, hh * 512:(hh + 1) * 512],
                            start=(j == 0), stop=(j == NT - 1))

                pend = None   # software pipeline: PV of j-1 runs while exp
                for j in range(NT):  # of j occupies the scalar engine
                    s_ps = ps.tile([128, 1024], F32, tag="s", name=f"s{j}")
                    jsl = slice(j * 128, (j + 1) * 128)
                    # heads A,B -> disjoint PE row groups run concurrently
                    for hh in range(2):
                        hsl = slice(hh * DH, (hh + 1) * DH)
                        nc.tensor.matmul(
                            s_ps[:, hh * 512:(hh + 1) * 512],
                            kTt[hsl, jsl], qT[hsl, isl],
                            start=True, stop=True)
                    expT = att.tile([128, 1024], BF16, tag="exp")
                    nc.scalar.activation(expT, s_ps, EXP, scale=SCALE)
                    if pend is not None:
                        emit_pv(j - 1, pend)
                    if j == 1 and pre is not None:
                        pre()
                    if fillers and j >= start_j and (j - start_j) % 2 == 0:
                        fillers.pop(0)()
                    pend = expT
                emit_pv(NT - 1, pend)
                while fillers:
                    fillers.pop(0)()
                # copy the PV accumulators to SBUF now (frees PSUM for the
                # next block); the rest of the normalization is deferred
                # into the next block via the returned closure
                if not last:
                    # mid-phase: reciprocal the denominator row in place,
                    # DMA it to partition 0 and pool-broadcast it in the
                    # next block - no PE/PSUM involvement, so the scores
                    # pipeline of the next block is never disturbed
                    o_sbs, rrows = [], []
                    for hh in range(2):
                        o_sb = norm_w.tile([DH + 1, 512], F32, tag=f"osb{hh}",
                                           name=f"osb{hh}")
                        nc.vector.tensor_copy(o_sb, o_ps[hh])
                        o_sbs.append(o_sb)
                    for hh in range(2):
                        rrow = norm_w.tile([1, 512], F32, tag=f"rr{hh}",
                                           name=f"rr{hh}")
                        nc.sync.dma_start(out=rrow,
                                          in_=o_sbs[hh][DH:DH + 1, :])
                        nc.vector.reciprocal_approx_fast(rrow, rrow)
                        rrows.append(rrow)

                    def norm_tail():
                        for hh in range(2):
                            bc = norm_w.tile([DH, 512], F32, tag=f"bc{hh}",
                                             name=f"bc{hh}")
                            nc.gpsimd.partition_broadcast(bc, rrows[hh])
                            if hh == 0:
                                nc.vector.tensor_mul(outT[p][iq][0:DH, :],
                                                     o_sbs[hh][0:DH, :], bc)
                            else:
                                tmpb = norm_w.tile([DH, 512], BF16,
                                                   tag="tmpb")
                                nc.vector.tensor_mul(tmpb,
                                                     o_sbs[hh][0:DH, :], bc)
                                nc.sync.dma_start(
                                    out=outT[p][iq][DH:2 * DH, :], in_=tmpb)
                    return norm_tail

                # last block: broadcast the denominator row with a K=1 ones
                # matmul on the PE (PSUM is free now) and keep everything on
                # the shortest chain; head 1's product goes to tmpb33 which
                # the final projection reads directly (no DMA)
                o_sbs = []
                for hh in range(2):
                    o_sb = norm_w.tile([DH + 1, 512], F32R, tag=f"osbr{hh}",
                                       name=f"osbr{hh}")
                    nc.vector.tensor_copy(o_sb, o_ps[hh])
                    o_sbs.append(o_sb)

                def norm_tail():
                    bc_ps = ps.tile([128, 1024], F32, tag="s", name="bc_ps")
                    for hh in range(2):
                        nc.tensor.matmul(
                            bc_ps[0:DH, hh * 512:(hh + 1) * 512],
                            ones_r[DH:DH + 1, :],
                            o_sbs[hh][DH:DH + 1, :],
                            start=True, stop=True)
                    for hh in range(2):
                        bc = norm_w.tile([DH, 512], F32, tag=f"bc{hh}",
                                         name=f"bc{hh}")
                        nc.vector.reciprocal_approx_fast(
                            bc, bc_ps[0:DH, hh * 512:(hh + 1) * 512])
                        if hh == 0:
                            nc.vector.tensor_mul(outT[p][iq][0:DH, :],
                                                 o_sbs[hh][0:DH, :], bc)
                        else:
                            nc.vector.tensor_mul(tmpb33,
                                                 o_sbs[hh][0:DH, :], bc)
                return norm_tail

            def q_chunk_fillers(m, c2):
                """Spread one q chunk (16 matmuls + copy + rope) over a
                block's 8 filler slots."""
                h = {}
                f = []
                for half in range(2):
                    for klo in (0, 3, 6):
                        khi = min(klo + 3, KT)
                        f.append(lambda m=m, c2=c2, half=half, klo=klo,
                                 khi=khi: qk_chunk_mm(m, c2, half, klo, khi, h))
                f.append(lambda: (qk_chunk_finish(m, c2, h, True),
                                  rope_rot(m, c2, h)))
                f.append(lambda: rope_finish(m, c2, h))
                return f

            # ---- emission order ----
            # upfront: k for both pairs (roped), v tiles 0-7, q pair0 chunk0.
            # chains before rots so the PE never waits on the PSUM->SBUF copy
            hold = {}
            for c2 in range(2):
                for p in range(2):
                    h = hold[(p, c2)] = {}
                    for half in range(2):
                        qk_chunk_mm(2 + p, c2, half, 0, KT, h)
                    qk_chunk_finish(2 + p, c2, h, False)
                if c2 == 0:
                    for tn in range(9):
                        v_tile(tn)
                for p in range(2):
                    rope_rot(2 + p, c2, hold[(p, c2)])
                    rope_finish(2 + p, c2, hold[(p, c2)])
            h = {}
            for half in range(2):
                qk_chunk_mm(0, 0, half, 0, KT, h)
            qk_chunk_finish(0, 0, h, False)
            rope_rot(0, 0, h)
            rope_finish(0, 0, h)

            # attention blocks with filler work in the PE slack
            nt = attention(0, 0, [lambda tn=tn: v_tile(tn)
                                  for tn in range(9, NT)])
            nt = attention(0, 1, q_chunk_fillers(0, 1), start_j=2, pre=nt)
            nt = attention(0, 2, q_chunk_fillers(1, 0), start_j=2, pre=nt)
            nt = attention(0, 3, q_chunk_fillers(1, 1), start_j=2, pre=nt)
            nt = attention(1, 0, pre=nt)
            nt = attention(1, 1, [lambda tn=tn: proj_tile(
                tn, nc.vector.tensor_copy) for tn in range(0, 4)],
                start_j=7, pre=nt)
            nt = attention(1, 2, [lambda tn=tn: proj_tile(
                tn, nc.vector.tensor_copy) for tn in range(4, 8)],
                start_j=7, pre=nt)
            nt = attention(1, 3, [lambda tn=tn: proj_tile(
                tn, nc.vector.tensor_copy) for tn in range(8, 12)],
                start_j=7, pre=nt, last=True)
            nt()
            for tn in range(12, NT):
                proj_last(tn, nc.vector.tensor_copy if tn % 2 else
                          nc.scalar.copy)
    nc.compile()
    _cache["nc"] = nc
    return nc


def kernel(x, w_qkv, w_out, b_out, _trace=False):
    import ml_dtypes
    from concourse.bass_utils import run_bass_kernel_spmd

    x = np.asarray(x, dtype=np.float32)
    w_qkv = np.asarray(w_qkv, dtype=np.float32)
    w_out = np.asarray(w_out, dtype=np.float32)
    b_out = np.asarray(b_out, dtype=np.float32)

    cos2, sin2 = _rope_tables()
    p2t = _p2t()

    in_maps = []
    for c in range(N_CORES):
        b, g = divmod(c, G)
        cols = []
        for blk in range(2):                      # q block, k block
            base = blk * H * DH + g * G * DH
            cols.append(w_qkv[:, base:base + G * DH])
        wqk_c = np.ascontiguousarray(np.concatenate(cols, axis=1))  # [DIM, 512]
        wv_c = np.ascontiguousarray(
            w_qkv[:, 2 * H * DH + g * G * DH: 2 * H * DH + (g + 1) * G * DH])
        wout_c = np.ascontiguousarray(
            w_out[g * G * DH:(g + 1) * G * DH, :]).astype(ml_dtypes.bfloat16)
        in_maps.append({
            "xT": np.ascontiguousarray(x[b].T).astype(ml_dtypes.bfloat16),
            "wqk": wqk_c.astype(ml_dtypes.bfloat16),
            "wv": wv_c.astype(ml_dtypes.bfloat16),
            "wout": wout_c,
            "cos2": cos2.astype(ml_dtypes.bfloat16),
            "sin2": sin2.astype(ml_dtypes.bfloat16),
            "p2t": p2t.astype(ml_dtypes.bfloat16),
            "onesr": np.ones((DH + 1, DH), dtype=np.float32),
        })

    nc = _build()
    res = run_bass_kernel_spmd(nc, in_maps, core_ids=list(range(N_CORES)),
                               trace=_trace)
    out = np.empty((B, N, DIM), dtype=np.float32)
    for b in range(B):
        acc = res.results[G * b]["part"].astype(np.float32)
        for g in range(1, G):
            acc += res.results[G * b + g]["part"].astype(np.float32)
        out[b] = acc + b_out
    if _trace:
        kernel.last_results = res
    return out


# revision 17
# speedup vs baseline: 1.0415x; 1.0415x over previous
"""Trainium2 Bass kernel for nn_Attention_35021163332119.

Full multi-head attention: qkv = x @ w_qkv; RoPE(q, k); softmax(q k^T / sqrt(dh)) v;
out = heads @ w_out + b_out.  B=2, N=2048, DIM=1024, H=16, DH=64.

Sharding: 8 cores = (batch b in {0,1}) x (head-group g in {0..3} of 4 heads).
Each core computes its 4 heads end-to-end plus the partial output projection
for its head-group's rows of w_out; the host sums the 4 partials per batch
and adds b_out.

On-core layout: x is host-transposed to xT [DIM, N] so the contraction dim
sits on SBUF partitions.  q,k are produced transposed ([dh, n], head pairs
stacked on 128 partitions) straight out of the QKV matmul; v is produced in
natural [n, dh] layout with an extra ones column, so the PV matmul (M=65)
also accumulates the softmax denominator in row 64.  RoPE's interleaved
pair-rotation is a 128x128 +/-1 permutation matmul on the PE plus two DVE
multiplies against cos/sin tables.

All matmuls run in bf16 (full PE stream rate; fp32r streams at half rate on
HW).  Scores accumulate fp32 in PSUM; exp runs on the Scalar engine reading
PSUM directly.  Per j-tile a single PSUM tile holds BOTH heads' scores side
by side so one ACTIVATE covers both heads and only one ps buffer is
consumed per j-step.

Schedule: the attention phase is Scalar(exp)-bound (~1.16us/j-step vs
~0.65us of PE work), so only K (both pairs), V tiles 0-7 and the first Q
chunk are computed up front; the remaining Q chunks, V tiles 8-15 and the
output projection are emitted as small "filler" pieces inside the attention
blocks' j loops where the PE has slack.  Inputs are loaded with a handful
of batched DMAs (multi-tile interleaved layout) because each DMA dispatch
costs ~600ns on the sync engine.
"""

import numpy as np

B, N, DIM, H, DH = 2, 2048, 1024, 16, 64
ROPE_BASE = 10000.0
SCALE = DH ** -0.5
N_CORES = 8
G = 4                 # heads per core
KT = DIM // 128       # contraction tiles
NT = N // 128         # sequence tiles

_cache = {}


def _rope_tables():
    inv_freq = (1.0 / (ROPE_BASE ** (np.arange(0, DH, 2, dtype=np.float32) / DH)))
    t = np.arange(N, dtype=np.float32)
    freqs = t[:, None] * inv_freq[None, :]          # [N, DH/2]
    freqs = np.repeat(freqs, 2, axis=-1)            # [N, DH] interleaved
    cosT = np.cos(freqs).T.astype(np.float32)       # [DH, N]
    sinT = np.sin(freqs).T.astype(np.float32)
    cos2 = np.concatenate([cosT, cosT], axis=0)     # [128, N] two heads stacked
    sin2 = np.concatenate([sinT, sinT], axis=0)
    return np.ascontiguousarray(cos2), np.ascontiguousarray(sin2)


def _p2t():
    # rot = P2 @ qT with P2 = blockdiag(P, P), P[2t, 2t+1] = -1, P[2t+1, 2t] = 1
    # matmul computes lhsT.T @ rhs, so pass P2.T
    p = np.zeros((DH, DH), dtype=np.float32)
    for t in range(DH // 2):
        p[2 * t, 2 * t + 1] = -1.0
        p[2 * t + 1, 2 * t] = 1.0
    p2 = np.zeros((128, 128), dtype=np.float32)
    p2[:DH, :DH] = p
    p2[DH:, DH:] = p
    return np.ascontiguousarray(p2.T)


def _build():
    if "nc" in _cache:
        return _cache["nc"]

    import concourse.mybir as mybir
    import concourse.tile as tile
    from concourse import bacc

    F32 = mybir.dt.float32
    F32R = mybir.dt.float32r
    BF16 = mybir.dt.bfloat16
    EXP = mybir.ActivationFunctionType.Exp

    nc = bacc.Bacc("TRN2", target_bir_lowering=False, debug=False)
    xT_d = nc.dram_tensor("xT", [DIM, N], BF16, kind="ExternalInput")
    wqk_d = nc.dram_tensor("wqk", [DIM, 4 * 128], BF16, kind="ExternalInput")
    wv_d = nc.dram_tensor("wv", [DIM, G * DH], BF16, kind="ExternalInput")
    wout_d = nc.dram_tensor("wout", [G * DH, DIM], BF16, kind="ExternalInput")
    cos_d = nc.dram_tensor("cos2", [128, N], BF16, kind="ExternalInput")
    sin_d = nc.dram_tensor("sin2", [128, N], BF16, kind="ExternalInput")
    p2t_d = nc.dram_tensor("p2t", [128, 128], BF16, kind="ExternalInput")
    ones_d = nc.dram_tensor("onesr", [DH + 1, DH], mybir.dt.float32,
                            kind="ExternalInput")
    part_d = nc.dram_tensor("part", [N, DIM], BF16, kind="ExternalOutput")

    with tile.TileContext(nc) as tc:
        with tc.tile_pool(name="persist", bufs=1) as persist, \
             tc.tile_pool(name="att", bufs=5) as att, \
             tc.tile_pool(name="norm_w", bufs=2) as norm_w, \
             tc.tile_pool(name="outp", bufs=3) as outp, \
             tc.tile_pool(name="xph", bufs=1) as xph, \
             tc.tile_pool(name="rope_w", bufs=2) as rope_w, \
             tc.tile_pool(name="ps", bufs=3, space="PSUM") as ps, \
             tc.tile_pool(name="pso", bufs=2, space="PSUM") as pso:

            # ---- persistent tiles ----
            qk_sb = [persist.tile([128, N], BF16, tag=f"qk{m}", name=f"qk{m}")
                     for m in range(4)]          # q01T, q23T, k01T, k23T
            v_aug = [persist.tile([128, G, DH + 1], BF16, tag=f"vaug{tn}",
                                  name=f"vaug{tn}")
                     for tn in range(NT)]        # per-j-tile for precise deps
            wout_sb = persist.tile([128, 2, DIM], BF16, tag="wo", name="wo")
            wout_hi2 = persist.tile([DH, DIM], BF16, tag="wohi", name="wohi")
            tmpb33 = persist.tile([DH, 512], BF16, tag="tmpb33", name="tmpb33")
            outT = [[persist.tile([128, 512], BF16, tag=f"outT{p}_{iq}",
                                  name=f"outT{p}_{iq}")
                     for iq in range(4)] for p in range(2)]

            # ---- phase-1 tiles (multi-tile interleaved so one DMA loads all
            # k-tiles of a column chunk) ----
            xT = xph.tile([128, KT, N], BF16, tag="xT", name="xT")
            wqk = xph.tile([128, KT, 4 * 128], BF16, tag="wqk", name="wqk")
            wv = xph.tile([128, KT, G * DH], BF16, tag="wv", name="wv")
            cos2 = xph.tile([128, N], BF16, tag="cos2")
            sin2 = xph.tile([128, N], BF16, tag="sin2")
            p2t = xph.tile([128, 128], BF16, tag="p2t")
            ones_r = xph.tile([DH + 1, DH], F32R, tag="ones_r")

            # ---- input DMA, priority order, batched ----
            xT_r = xT_d.ap().rearrange("(t p) n -> p t n", p=128)
            wqk_r = wqk_d.ap().rearrange("(t p) m -> p t m", p=128)
            nc.sync.dma_start(out=wqk[:, :, 256:512], in_=wqk_r[:, :, 256:512])
            nc.sync.dma_start(out=xT[:, 0:4, 0:512], in_=xT_r[:, 0:4, 0:512])
            nc.sync.dma_start(out=xT[:, 4:8, 0:512], in_=xT_r[:, 4:8, 0:512])
            nc.sync.dma_start(out=xT[:, :, 512:1024], in_=xT_r[:, :, 512:1024])
            nc.sync.dma_start(
                out=wv, in_=wv_d.ap().rearrange("(t p) m -> p t m", p=128))
            nc.sync.dma_start(out=cos2, in_=cos_d.ap())
            nc.sync.dma_start(out=sin2, in_=sin_d.ap())
            nc.sync.dma_start(out=p2t, in_=p2t_d.ap())
            nc.sync.dma_start(out=ones_r, in_=ones_d.ap().bitcast(F32R))
            nc.sync.dma_start(out=wqk[:, :, 0:256], in_=wqk_r[:, :, 0:256])
            nc.sync.dma_start(out=xT[:, :, 1024:1536], in_=xT_r[:, :, 1024:1536])
            nc.sync.dma_start(out=xT[:, :, 1536:2048], in_=xT_r[:, :, 1536:2048])
            nc.sync.dma_start(
                out=wout_sb, in_=wout_d.ap().rearrange("(t p) m -> p t m", p=128))
            nc.sync.dma_start(
                out=wout_hi2,
                in_=wout_d.ap().rearrange("(t p) m -> p t m", p=64)[:, 3, :])
            for tn in range(NT):
                nc.vector.memset(v_aug[tn][:, :, DH:DH + 1], 1.0)

            # ---- emitters ----
            def qk_chunk_mm(m, c2, half, klo, khi, holder):
                """Piece of the [128,1024] QKV chain for tile m, chunk c2:
                k-range [klo,khi) of the `half` 512-accumulation."""
                if holder.get("t") is None:
                    holder["t"] = ps.tile([128, 1024], F32, tag="s",
                                          name=f"mm_qk{m}_{c2}")
                mm_ps = holder["t"]
                hsl = slice(half * 512, (half + 1) * 512)
                csl = slice(c2 * 1024 + half * 512, c2 * 1024 + (half + 1) * 512)
                for k in range(klo, khi):
                    nc.tensor.matmul(
                        mm_ps[:, hsl],
                        wqk[:, k, m * 128:(m + 1) * 128],
                        xT[:, k, csl],
                        start=(k == 0), stop=(k == KT - 1))

            def qk_chunk_finish(m, c2, holder, use_vector):
                csl = slice(c2 * 1024, (c2 + 1) * 1024)
                if use_vector:
                    nc.vector.tensor_copy(qk_sb[m][:, csl], holder["t"])
                else:
                    nc.scalar.copy(qk_sb[m][:, csl], holder["t"])
                holder["t"] = None

            def rope_rot(m, c2, holder):
                """rot = P2 @ qk chunk -> PSUM."""
                holder["t"] = ps.tile([128, 1024], F32, tag="s",
                                      name=f"mm_rot{m}_{c2}")
                for half in range(2):
                    csl = slice(c2 * 1024 + half * 512,
                                c2 * 1024 + (half + 1) * 512)
                    nc.tensor.matmul(
                        holder["t"][:, half * 512:(half + 1) * 512],
                        p2t, qk_sb[m][:, csl],
                        start=True, stop=True)

            def rope_finish(m, c2, holder):
                csl = slice(c2 * 1024, (c2 + 1) * 1024)
                tmp = rope_w.tile([128, 1024], BF16, tag="ropetmp")
                nc.vector.tensor_mul(tmp, holder["t"], sin2[:, csl])
                nc.vector.tensor_mul(qk_sb[m][:, csl], qk_sb[m][:, csl],
                                     cos2[:, csl])
                nc.vector.tensor_add(qk_sb[m][:, csl], qk_sb[m][:, csl], tmp)
                holder["t"] = None

            def v_tile(tn):
                mm_ps = ps.tile([128, 1024], F32, tag="s", name=f"mm_v{tn}")
                for k in range(KT):
                    nc.tensor.matmul(
                        mm_ps[:, 0:G * DH],
                        xT[:, k, tn * 128:(tn + 1) * 128],
                        wv[:, k, :],
                        start=(k == 0), stop=(k == KT - 1))
                nc.vector.tensor_copy(
                    v_aug[tn][:, :, 0:DH],
                    mm_ps[:, 0:G * DH].rearrange("p (h d) -> p h d", h=G))

            def proj_tile(tn, copy_eng):
                nsl = slice((tn % 4) * 128, (tn % 4) * 128 + 128)
                iq = tn // 4
                f_ps = ps.tile([128, 1024], F32, tag="s", name=f"f_ps{tn}")
                for c2 in range(2):
                    c2sl = slice(c2 * 512, (c2 + 1) * 512)
                    for kk in range(2):
                        nc.tensor.matmul(
                            f_ps[:, c2sl],
                            outT[kk][iq][:, nsl], wout_sb[:, kk, c2sl],
                            start=(kk == 0), stop=(kk == 1))
                out_sb = outp.tile([128, DIM], BF16, tag="osb")
                copy_eng(out_sb, f_ps)
                nc.sync.dma_start(
                    out=part_d.ap().rearrange("(t p) m -> t p m", p=128)[tn],
                    in_=out_sb)

            def proj_last(tn, copy_eng):
                nsl = slice((tn % 4) * 128, (tn % 4) * 128 + 128)
                f_ps = ps.tile([128, 1024], F32, tag="s", name=f"f_ps{tn}")
                for c2 in range(2):
                    c2sl = slice(c2 * 512, (c2 + 1) * 512)
                    nc.tensor.matmul(
                        f_ps[:, c2sl], outT[0][3][:, nsl],
                        wout_sb[:, 0, c2sl], start=True, stop=False)
                    nc.tensor.matmul(
                        f_ps[:, c2sl], outT[1][3][0:DH, nsl],
                        wout_sb[0:DH, 1, c2sl], start=False, stop=False)
                    nc.tensor.matmul(
                        f_ps[:, c2sl], tmpb33[:, nsl],
                        wout_hi2[:, c2sl], start=False, stop=True)
                out_sb = outp.tile([128, DIM], BF16, tag="osb")
                copy_eng(out_sb, f_ps)
                nc.sync.dma_start(
                    out=part_d.ap().rearrange("(t p) m -> t p m", p=128)[tn],
                    in_=out_sb)

            def attention(p, iq, fillers=None, start_j=1, pre=None,
                          last=False):
                """One (head-pair, i-quarter of 512) block.  Per j-tile a
                single PSUM tile holds BOTH heads' scores side by side
                ([j=128, head0 i | head1 i]) so one ACTIVATE covers both
                heads and only one ps buffer is consumed per j-step - the
                scores pipeline keeps a 2-step cushion even when a filler
                chain occupies a third buffer.  `fillers` is a list of
                zero-arg closures emitting small PE pieces into the PE
                slack (the block is Scalar-bound); one is drained every
                second j-step starting at `start_j` (delay it if the filler
                depends on the previous block's normalization).  `pre` is
                the previous block's norm tail, emitted at j==1 so the PE
                does not stall on it at the block boundary.  Returns this
                block's norm tail closure."""
                fillers = list(fillers or [])
                qT = qk_sb[p]
                kTt = qk_sb[2 + p]
                i0 = iq * 512
                isl = slice(i0, i0 + 512)
                o_ps = [pso.tile([DH + 1, 512], F32, tag="o", name=f"o{hh}")
                        for hh in range(2)]

                def emit_pv(j, expT):
                    for hh in range(2):
                        nc.tensor.matmul(
                            o_ps[hh],
                            v_aug[j][:, 2 * p + hh, :],
                            expT[:, hh * 512:(hh + 1) * 512],
                            start=(j == 0), stop=(j == NT - 1))

                pend = None   # software pipeline: PV of j-1 runs while exp
                for j in range(NT):  # of j occupies the scalar engine
                    s_ps = ps.tile([128, 1024], F32, tag="s", name=f"s{j}")
                    jsl = slice(j * 128, (j + 1) * 128)
                    # heads A,B -> disjoint PE row groups run concurrently
                    for hh in range(2):
                        hsl = slice(hh * DH, (hh + 1) * DH)
                        nc.tensor.matmul(
                            s_ps[:, hh * 512:(hh + 1) * 512],
                            kTt[hsl, jsl], qT[hsl, isl],
                            start=True, stop=True)
                    expT = att.tile([128, 1024], BF16, tag="exp")
                    nc.scalar.activation(expT, s_ps, EXP, scale=SCALE)
                    if pend is not None:
                        emit_pv(j - 1, pend)
                    if j == 1 and pre is not None:
                        pre()
                    if fillers and j >= start_j and (j - start_j) % 2 == 0:
                        fillers.pop(0)()
                    pend = expT
                emit_pv(NT - 1, pend)
                while fillers:
                    fillers.pop(0)()
                # copy the PV accumulators to SBUF now (frees PSUM for the
                # next block); the rest of the normalization is deferred
                # into the next block via the returned closure
                if not last:
                    # mid-phase: reciprocal the denominator row in place,
                    # DMA it to partition 0 and pool-broadcast it in the
                    # next block - no PE/PSUM involvement, so the scores
                    # pipeline of the next block is never disturbed
                    o_sbs, rrows = [], []
                    for hh in range(2):
                        o_sb = norm_w.tile([DH + 1, 512], F32, tag=f"osb{hh}",
                                           name=f"osb{hh}")
                        nc.vector.tensor_copy(o_sb, o_ps[hh])
                        o_sbs.append(o_sb)
                    for hh in range(2):
                        rrow = norm_w.tile([1, 512], F32, tag=f"rr{hh}",
                                           name=f"rr{hh}")
                        nc.sync.dma_start(out=rrow,
                                          in_=o_sbs[hh][DH:DH + 1, :])
                        nc.vector.reciprocal_approx_fast(rrow, rrow)
                        rrows.append(rrow)

                    def norm_tail():
                        for hh in range(2):
                            bc = norm_w.tile([DH, 512], F32, tag=f"bc{hh}",
                                             name=f"bc{hh}")
                            nc.gpsimd.partition_broadcast(bc, rrows[hh])
                            if hh == 0:
                                nc.vector.tensor_mul(outT[p][iq][0:DH, :],
                                                     o_sbs[hh][0:DH, :], bc)
                            else:
                                tmpb = norm_w.tile([DH, 512], BF16,
                                                   tag="tmpb")
                                nc.vector.tensor_mul(tmpb,
                                                     o_sbs[hh][0:DH, :], bc)
                                nc.sync.dma_start(
                                    out=outT[p][iq][DH:2 * DH, :], in_=tmpb)
                    return norm_tail

                # last block: broadcast the denominator row with a K=1 ones
                # matmul on the PE (PSUM is free now) and keep everything on
                # the shortest chain; head 1's product goes to tmpb33 which
                # the final projection reads directly (no DMA)
                o_sbs = []
                for hh in range(2):
                    o_sb = norm_w.tile([DH + 1, 512], F32R, tag=f"osbr{hh}",
                                       name=f"osbr{hh}")
                    nc.vector.tensor_copy(o_sb, o_ps[hh])
                    o_sbs.append(o_sb)

                def norm_tail():
                    bc_ps = ps.tile([128, 1024], F32, tag="s", name="bc_ps")
                    for hh in range(2):
                        nc.tensor.matmul(
                            bc_ps[0:DH, hh * 512:(hh + 1) * 512],
                            ones_r[DH:DH + 1, :],
                            o_sbs[hh][DH:DH + 1, :],
                            start=True, stop=True)
                    for hh in range(2):
                        bc = norm_w.tile([DH, 512], F32, tag=f"bc{hh}",
                                         name=f"bc{hh}")
                        nc.vector.reciprocal_approx_fast(
                            bc, bc_ps[0:DH, hh * 512:(hh + 1) * 512])
                        if hh == 0:
                            nc.vector.tensor_mul(outT[p][iq][0:DH, :],
                                                 o_sbs[hh][0:DH, :], bc)
                        else:
                            nc.vector.tensor_mul(tmpb33,
                                                 o_sbs[hh][0:DH, :], bc)
                return norm_tail

            def q_chunk_fillers(m, c2):
                """Spread one q chunk (16 matmuls + copy + rope) over a
                block's 8 filler slots."""
                h = {}
                f = []
                for half in range(2):
                    for klo in (0, 3, 6):
                        khi = min(klo + 3, KT)
                        f.append(lambda m=m, c2=c2, half=half, klo=klo,
                                 khi=khi: qk_chunk_mm(m, c2, half, klo, khi, h))
                f.append(lambda: (qk_chunk_finish(m, c2, h, True),
                                  rope_rot(m, c2, h)))
                f.append(lambda: rope_finish(m, c2, h))
                return f

            # ---- emission order ----
            # upfront: k for both pairs (roped), v tiles 0-7, q pair0 chunk0.
            # chains before rots so the PE never waits on the PSUM->SBUF copy
            hold = {}
            for c2 in range(2):
                for p in range(2):
                    h = hold[(p, c2)] = {}
                    for half in range(2):
                        qk_chunk_mm(2 + p, c2, half, 0, KT, h)
                    qk_chunk_finish(2 + p, c2, h, False)
                if c2 == 0:
                    for tn in range(4):
                        v_tile(tn)
                for p in range(2):
                    rope_rot(2 + p, c2, hold[(p, c2)])
                    rope_finish(2 + p, c2, hold[(p, c2)])
            h = {}
            for half in range(2):
                qk_chunk_mm(0, 0, half, 0, KT, h)
            qk_chunk_finish(0, 0, h, False)
            rope_rot(0, 0, h)
            rope_finish(0, 0, h)
            # v tiles 4-9 after the q00 rot: the PE stays busy (and clocked
            # up) while the DVE applies the q00 rope, so attention(0,0)
            # opens with no PE gap
            for tn in range(4, 10):
                v_tile(tn)

            # attention blocks with filler work in the PE slack
            nt = attention(0, 0, [lambda tn=tn: v_tile(tn)
                                  for tn in range(10, NT)])
            nt = attention(0, 1, q_chunk_fillers(0, 1), start_j=2, pre=nt)
            nt = attention(0, 2, q_chunk_fillers(1, 0), start_j=2, pre=nt)
            nt = attention(0, 3, q_chunk_fillers(1, 1), start_j=2, pre=nt)
            nt = attention(1, 0, pre=nt)
            nt = attention(1, 1, [lambda tn=tn: proj_tile(
                tn, nc.vector.tensor_copy) for tn in range(0, 4)],
                start_j=7, pre=nt)
            nt = attention(1, 2, [lambda tn=tn: proj_tile(
                tn, nc.vector.tensor_copy) for tn in range(4, 8)],
                start_j=7, pre=nt)
            nt = attention(1, 3, [lambda tn=tn: proj_tile(
                tn, nc.vector.tensor_copy) for tn in range(8, 12)],
                start_j=7, pre=nt, last=True)
            nt()
            for tn in range(12, NT):
                proj_last(tn, nc.vector.tensor_copy if tn % 2 else
                          nc.scalar.copy)
    nc.compile()
    _cache["nc"] = nc
    return nc


def kernel(x, w_qkv, w_out, b_out, _trace=False):
    import ml_dtypes
    from concourse.bass_utils import run_bass_kernel_spmd

    x = np.asarray(x, dtype=np.float32)
    w_qkv = np.asarray(w_qkv, dtype=np.float32)
    w_out = np.asarray(w_out, dtype=np.float32)
    b_out = np.asarray(b_out, dtype=np.float32)

    cos2, sin2 = _rope_tables()
    p2t = _p2t()

    in_maps = []
    for c in range(N_CORES):
        b, g = divmod(c, G)
        cols = []
        for blk in range(2):                      # q block, k block
            base = blk * H * DH + g * G * DH
            cols.append(w_qkv[:, base:base + G * DH])
        wqk_c = np.ascontiguousarray(np.concatenate(cols, axis=1))  # [DIM, 512]
        wv_c = np.ascontiguousarray(
            w_qkv[:, 2 * H * DH + g * G * DH: 2 * H * DH + (g + 1) * G * DH])
        wout_c = np.ascontiguousarray(
            w_out[g * G * DH:(g + 1) * G * DH, :]).astype(ml_dtypes.bfloat16)
        in_maps.append({
            "xT": np.ascontiguousarray(x[b].T).astype(ml_dtypes.bfloat16),
            "wqk": wqk_c.astype(ml_dtypes.bfloat16),
            "wv": wv_c.astype(ml_dtypes.bfloat16),
            "wout": wout_c,
            "cos2": cos2.astype(ml_dtypes.bfloat16),
            "sin2": sin2.astype(ml_dtypes.bfloat16),
            "p2t": p2t.astype(ml_dtypes.bfloat16),
            "onesr": np.ones((DH + 1, DH), dtype=np.float32),
        })

    nc = _build()
    res = run_bass_kernel_spmd(nc, in_maps, core_ids=list(range(N_CORES)),
                               trace=_trace)
    out = np.empty((B, N, DIM), dtype=np.float32)
    for b in range(B):
        acc = res.results[G * b]["part"].astype(np.float32)
        for g in range(1, G):
            acc += res.results[G * b + g]["part"].astype(np.float32)
        out[b] = acc + b_out
    if _trace:
        kernel.last_results = res
    return out
